# revision 6
# baseline (speedup 1.0000x reference)
"""Trainium2 Bass kernel: MultiHeadCrossAttentionWithBias (v2, all-fp16 PE).

Reference computation (per batch b):
  q_u = scale*(u_enc @ wq + wq_b); k/v from e_enc (and vice versa)
  ue_w = softmax(q_u k_e^T + bpp + mask*-inf); u_ctx = ue_w @ v_e
  u_update = u_ctx @ wo + wo_b                     (same mirrored for e)

Sharding: 8 independent units (batch b, direction d); core i = (d, b)
handles one unit end-to-end; no collectives.

Measured-hw design notes:
  - PE runs 1 col/cycle @2.4GHz only in a uniform-dtype stream; mixing
    fp32r and 16-bit matmuls costs 2-3x. Everything on the PE is fp16.
  - exp(S+CB) = exp(S) * exp(CB): ECB = exp(bpp_w*bppm + bpp_b) built
    on-device once per kc chunk (8 ACT ops); mask pre-folded on host as
    bppm = where(mask, bpp, -6e4) so masked entries exp to exactly 0.
  - Per (h, kc): QK (PE, fp16) -> exp (ACT, psum->f16) -> E = er*ECB
    (DVE tensor_tensor, fp16 2x mode) -> PV (PE, fp16, lag 2).
  - Denominator via ones column in v_aug; odd heads use [1|v] layout and
    a partition-63 psum offset so ctx lands on partitions 64..127 with
    no cross-partition copies anywhere.
  - Normalize: reciprocal_approx_fast on the den psum row (DVE), DRAM
    bounce broadcast (gpsimd DMA casts f32->f16), gpsimd stt multiply
    straight from psum into ctxn (f16).
  - Projections for pair pc are emitted between attention pairs so the
    ACT exp stream starts ~10us in instead of after all projections.
"""

import numpy as np
from contextlib import ExitStack

import concourse.bass as bass
import concourse.tile as tile
import concourse.bacc as bacc
import concourse.mybir as mybir
from concourse import bass_utils

F32 = mybir.dt.float32
F16 = mybir.dt.float16
AF = mybir.ActivationFunctionType
ALU = mybir.AluOpType

B, L, D, H, HD = 4, 1024, 512, 8, 64
P = 128
FH = H * HD            # 512
SCALE = 1.0 / np.sqrt(HD)
NEG = -60000.0         # f16-safe -inf for masked logits
N_CORES = 8
LAG = 2


def bcast_ap(dram_ap, parts):
    """Partition-step-0 broadcast AP over a DRAM row."""
    return bass.AP(tensor=dram_ap.tensor, offset=dram_ap.offset,
                   ap=[[0, parts]] + list(dram_ap.ap))


def build_module():
    nc = bacc.Bacc("TRN2", target_bir_lowering=False, debug=False)

    encQT_d = nc.dram_tensor("encQT", [D, L], F16, kind="ExternalInput")
    encKT_d = nc.dram_tensor("encKT", [D, L], F16, kind="ExternalInput")
    wq_d = nc.dram_tensor("wq", [D, FH], F16, kind="ExternalInput")
    wk_d = nc.dram_tensor("wk", [D, FH], F16, kind="ExternalInput")
    wv_d = nc.dram_tensor("wv", [D, FH], F16, kind="ExternalInput")
    wo_d = nc.dram_tensor("wo", [FH, D], F16, kind="ExternalInput")
    bppm_d = nc.dram_tensor("bppm", [L, L], F16, kind="ExternalInput")
    wqb_d = nc.dram_tensor("wqb", [FH], F32, kind="ExternalInput")
    wkb_d = nc.dram_tensor("wkb", [FH], F32, kind="ExternalInput")
    wvb_d = nc.dram_tensor("wvb", [FH], F32, kind="ExternalInput")
    wob_d = nc.dram_tensor("wob", [1, D], F16, kind="ExternalInput")
    bppw_d = nc.dram_tensor("bppw", [1, 1], F32, kind="ExternalInput")
    bppb_d = nc.dram_tensor("bppb", [1, 1], F32, kind="ExternalInput")
    out_d = nc.dram_tensor("out", [L, D], F32, kind="ExternalOutput")
    den_d = nc.dram_tensor("den_scratch", [H, L], F32, kind="Internal")

    with tile.TileContext(nc) as tc, ExitStack() as ctx:
        const = ctx.enter_context(tc.tile_pool(name="const", bufs=1))
        qkT_p = ctx.enter_context(tc.tile_pool(name="qkT", bufs=8))
        v_p = ctx.enter_context(tc.tile_pool(name="v", bufs=8))
        ecb_p = ctx.enter_context(tc.tile_pool(name="ecb", bufs=8))
        wo_p = ctx.enter_context(tc.tile_pool(name="wo", bufs=8))
        ctxn_p = ctx.enter_context(tc.tile_pool(name="ctxn", bufs=8))
        rb_p = ctx.enter_context(tc.tile_pool(name="rb", bufs=2))
        rcp_p = ctx.enter_context(tc.tile_pool(name="rcp", bufs=2))
        enc_p = ctx.enter_context(tc.tile_pool(name="enc", bufs=8))
        w_p = ctx.enter_context(tc.tile_pool(name="wqkv", bufs=12))

        s_ps = tc.alloc_tile_pool(name="s_ps", bufs=2, space="PSUM")
        c_ps = tc.alloc_tile_pool(name="c_ps", bufs=1, space="PSUM")

        # ---- tiny bias prep (gpsimd queue) ----
        bw_col = const.tile([P, 1], F32)
        nc.gpsimd.dma_start(bw_col[:], bcast_ap(bppw_d.ap()[0:1, :], P))
        bb_col = const.tile([P, 1], F32)
        nc.gpsimd.dma_start(bb_col[:], bcast_ap(bppb_d.ap()[0:1, :], P))
        wqb_c = const.tile([P, 4], F32)
        nc.gpsimd.dma_start(wqb_c[:], wqb_d.ap().rearrange("(c p) -> p c", p=P))
        wkb_c = const.tile([P, 4], F32)
        nc.gpsimd.dma_start(wkb_c[:], wkb_d.ap().rearrange("(c p) -> p c", p=P))
        wvb_bc = const.tile([P, FH], F32)
        nc.gpsimd.dma_start(wvb_bc[:], bcast_ap(wvb_d.ap(), P))
        ones1 = const.tile([1, P], F16)
        nc.vector.memset(ones1[:], 1.0)
        wob_t = const.tile([1, D], F16)
        nc.gpsimd.dma_start(wob_t[:], wob_d.ap())

        # ---- weight / encoder loads in first-use order ----
        wq_t, wk_t, wv_t, eq, ek = [], [], [], [], []
        for dc in range(4):
            t = w_p.tile([P, FH], F16, tag="w", name=f"wq{dc}")
            nc.sync.dma_start(t[:], wq_d.ap()[dc * P:(dc + 1) * P, :])
            wq_t.append(t)
        for dc in range(4):
            t = enc_p.tile([P, L], F16, tag="enc", name=f"eq{dc}")
            nc.sync.dma_start(t[:], encQT_d.ap()[dc * P:(dc + 1) * P, :])
            eq.append(t)
        for dc in range(4):
            t = w_p.tile([P, FH], F16, tag="w", name=f"wk{dc}")
            nc.sync.dma_start(t[:], wk_d.ap()[dc * P:(dc + 1) * P, :])
            wk_t.append(t)
        for dc in range(4):
            t = enc_p.tile([P, L], F16, tag="enc", name=f"ek{dc}")
            nc.sync.dma_start(t[:], encKT_d.ap()[dc * P:(dc + 1) * P, :])
            ek.append(t)
        for dc in range(4):
            t = w_p.tile([P, FH], F16, tag="w", name=f"wv{dc}")
            nc.sync.dma_start(t[:], wv_d.ap()[dc * P:(dc + 1) * P, :])
            wv_t.append(t)

        # ---- ECB = exp(bpp_w * bppm + bpp_b), 8 chunks [128, L] ----
        ecb = []
        bppm_pool = tc.alloc_tile_pool(name="bppm", bufs=3)
        for kc in range(8):
            bt = bppm_pool.tile([P, L], F16, tag="bppm", name=f"bppm{kc}")
            nc.sync.dma_start(bt[:], bppm_d.ap()[kc * P:(kc + 1) * P, :])
            et = ecb_p.tile([P, L], F16, tag="ecb", name=f"ecb{kc}")
            nc.scalar.activation(et[:], bt[:], AF.Exp,
                                 bias=bb_col[:, 0:1], scale=bw_col[:, 0:1])
            ecb.append(et)
        bppm_pool.release()

        qT, kT = [None] * 4, [None] * 4
        va = [None] * 8

        def proj_qk(pc):
            """Project q and k for head-pair pc -> qT[pc], kT[pc] (f16)."""
            for which, w_t, enc_t, outl, bias in (
                ("q", wq_t, eq, qT, wqb_c), ("k", wk_t, ek, kT, wkb_c),
            ):
                ps = s_ps.tile([P, L], F32, tag="s", name=f"ps_{which}{pc}")
                for sh in range(2):
                    for dc in range(4):
                        nc.tensor.matmul(
                            ps[:, sh * 512:(sh + 1) * 512],
                            w_t[dc][:, pc * P:(pc + 1) * P],
                            enc_t[dc][:, sh * 512:(sh + 1) * 512],
                            start=(dc == 0), stop=(dc == 3))
                o = qkT_p.tile([P, L], F16, tag="qkT", name=f"{which}T{pc}")
                nc.scalar.activation(o[:], ps[:], AF.Identity,
                                     bias=bias[:, pc:pc + 1], scale=1.0)
                outl[pc] = o

        def proj_v(sc):
            """Project v chunk sc -> va[sc] [128, 4*130] f16 (+ones cols)."""
            ps = s_ps.tile([P, L], F32, tag="s", name=f"ps_v{sc}")
            for dc in range(4):
                nc.tensor.matmul(ps[:, 0:512], ek[dc][:, sc * P:(sc + 1) * P],
                                 wv_t[dc][:], start=(dc == 0), stop=(dc == 3))
            t = v_p.tile([P, H * 65], F16, tag="v", name=f"v{sc}")
            vg = t[:].rearrange("p (g c) -> p g c", c=65)
            pg = ps[:, 0:512].rearrange("p (g c) -> p g c", c=64)
            wg = wvb_bc[:].rearrange("p (g c) -> p g c", c=64)
            nc.vector.scalar_tensor_tensor(
                vg[:, :, 0:64], pg[:], 1.0, wg[:], ALU.bypass, ALU.add)
            nc.vector.memset(vg[:, :, 64:65], 1.0)
            va[sc] = t

        def attention_head(h, er_p, ee_p):
            pc, odd = h // 2, h % 2
            o = odd * HD
            ct = [c_ps.tile([65, 512], F32, tag=f"c{qh}{odd}",
                            name=f"ct{h}_{qh}") for qh in range(2)]
            es = {}
            for t in range(8 + LAG):
                if t < 8:
                    sp = s_ps.tile([P, L], F32, tag="s", name=f"s{h}_{t}")
                    for qh in range(2):
                        nc.tensor.matmul(
                            sp[:, qh * 512:(qh + 1) * 512],
                            kT[pc][o:o + HD, t * P:(t + 1) * P],
                            qT[pc][o:o + HD, qh * 512:(qh + 1) * 512],
                            start=True, stop=True)
                    er = er_p.tile([P, L], F16, tag="er", name=f"er{h}_{t}")
                    nc.scalar.activation(er[:], sp[:], AF.Exp)
                    ee = ee_p.tile([P, L], F16, tag="ee", name=f"ee{h}_{t}")
                    nc.vector.tensor_tensor(ee[:], er[:], ecb[t][:], ALU.mult)
                    es[t] = ee
                if t >= LAG:
                    kp = t - LAG
                    lhs = va[kp][:].rearrange("p (g c) -> p g c", c=65)[
                        :, h, :]
                    for qh in range(2):
                        nc.tensor.matmul(ct[qh][:], lhs,
                                         es[kp][:, qh * 512:(qh + 1) * 512],
                                         start=(kp == 0), stop=(kp == 7))
            # normalize: den row -> SBUF -> rcp -> DRAM bounce -> DVE stt
            dsb = rcp_p.tile([65, L], F32, tag="dsb", name=f"dsb{h}")
            rcp = rcp_p.tile([65, L], F32, tag="rcp", name=f"rcp{h}")
            for qh in range(2):
                nc.scalar.copy(dsb[64:65, qh * 512:(qh + 1) * 512],
                               ct[qh][64:65, :])
            nc.vector.reciprocal(rcp[64:65, :], dsb[64:65, :])
            nc.sync.dma_start(den_d.ap()[h:h + 1, :], rcp[64:65, :])
            rbt = rb_p.tile([HD, L], F32, tag="rb", name=f"rb{h}")
            cxt = ctxn_p.tile([HD, L], F16, tag="ctxn", name=f"ctxn{h}")
            nc.gpsimd.dma_start(rbt[:],
                                bcast_ap(den_d.ap()[h:h + 1, :], HD))
            for qh in range(2):
                nc.vector.scalar_tensor_tensor(
                    cxt[:, qh * 512:(qh + 1) * 512], ct[qh][0:64, :], 1.0,
                    rbt[:, qh * 512:(qh + 1) * 512], ALU.bypass, ALU.mult)
            return cxt

        # ---- pipeline: pair0 projections, v, then per-pair attention ----
        ctxn = [None] * 8
        with tc.tile_pool(name="er", bufs=2) as er_p, \
             tc.tile_pool(name="ee", bufs=4) as ee_p:
            proj_qk(0)
            for sc in range(8):
                proj_v(sc)
            for pc in range(4):
                if pc > 0:
                    proj_qk(pc)
                for h in (2 * pc, 2 * pc + 1):
                    ctxn[h] = attention_head(h, er_p, ee_p)

        # ---- wo loads (late): per-head [64, D] tiles ----
        wo_t = []
        for h in range(H):
            t = wo_p.tile([HD, D], F16, tag="wo", name=f"wo{h}")
            nc.sync.dma_start(t[:], wo_d.ap()[h * HD:(h + 1) * HD, :])
            wo_t.append(t)

        # ---- output projection (fp16, bias via K=1 ones row) ----
        c_ps.release()
        s_ps.release()
        ps_o = tc.alloc_tile_pool(name="ps_o", bufs=8, space="PSUM")
        with tc.tile_pool(name="outp", bufs=3) as out_p:
            o_ps = [ps_o.tile([P, D], F32, tag="ps_o", name=f"o_ps{st}")
                    for st in range(8)]
            for st in range(8):
                nc.tensor.matmul(o_ps[st][:], ones1[0:1, :],
                                 wob_t[0:1, :], start=True, stop=False)
            for h in range(H):
                for st in range(8):
                    nc.tensor.matmul(o_ps[st][:],
                                     ctxn[h][:, st * P:(st + 1) * P],
                                     wo_t[h][:],
                                     start=False, stop=(h == 7))
            for st in range(8):
                o_t = out_p.tile([P, D], F32, tag="out", name=f"out{st}")
                nc.vector.tensor_copy(o_t[:], o_ps[st][:])
                nc.sync.dma_start(out_d.ap()[st * P:(st + 1) * P, :], o_t[:])
        ps_o.release()

    nc.compile()
    return nc


def shard_inputs(u_enc, e_enc, logit_bpp, ue_mask, eu_mask,
                 wq_k, wq_b, wk_k, wk_b, wv_k, wv_b, wo_k, wo_b,
                 bpp_w, bpp_b):
    """Build the 8 per-core input maps (layout + f16 rounding only)."""
    u_enc = np.asarray(u_enc, np.float32)
    e_enc = np.asarray(e_enc, np.float32)
    bpp = np.asarray(logit_bpp, np.float32)
    ue_m = np.asarray(ue_mask).astype(bool)
    eu_m = np.asarray(eu_mask).astype(bool)
    com = dict(
        wq=(np.asarray(wq_k, np.float32).reshape(D, FH) * SCALE
            ).astype(np.float16),
        wk=np.asarray(wk_k, np.float32).reshape(D, FH).astype(np.float16),
        wv=np.asarray(wv_k, np.float32).reshape(D, FH).astype(np.float16),
        wo=np.asarray(wo_k, np.float32).reshape(FH, D).astype(np.float16),
        wqb=(np.asarray(wq_b, np.float32).reshape(FH) * SCALE).copy(),
        wkb=np.asarray(wk_b, np.float32).reshape(FH).copy(),
        wvb=np.asarray(wv_b, np.float32).reshape(FH).copy(),
        wob=np.asarray(wo_b, np.float32).reshape(1, D).astype(np.float16),
        bppw=np.asarray(bpp_w, np.float32).reshape(1, 1).copy(),
        bppb=np.asarray(bpp_b, np.float32).reshape(1, 1).copy(),
    )
    uT = [u_enc[b].T.astype(np.float16) for b in range(B)]
    eT = [e_enc[b].T.astype(np.float16) for b in range(B)]
    bppT = np.ascontiguousarray(bpp.T)
    in_maps = []
    for i in range(N_CORES):
        d, b = divmod(i, B)
        if d == 0:      # u queries, e keys -> u_update[b]
            bm = np.where(ue_m[b, 0].T, bppT, NEG).astype(np.float16)
            m = dict(encQT=uT[b], encKT=eT[b], bppm=bm)
        else:           # e queries, u keys -> e_update[b]
            bm = np.where(eu_m[b, 0].T, bpp, NEG).astype(np.float16)
            m = dict(encQT=eT[b], encKT=uT[b], bppm=bm)
        m.update(com)
        in_maps.append(m)
    return in_maps


_NC = None


def kernel(**inputs):
    global _NC
    if _NC is None:
        _NC = build_module()
    in_maps = shard_inputs(**inputs)
    res = bass_utils.run_bass_kernel_spmd(
        _NC, in_maps, core_ids=list(range(N_CORES)))
    u_update = np.stack([res.results[b]["out"] for b in range(B)])
    e_update = np.stack([res.results[B + b]["out"] for b in range(B)])
    return u_update, e_update


if __name__ == "__main__":
    # single-core CoreSim check of one (direction, batch) unit
    from concourse.bass_interp import CoreSim

    rng = np.random.default_rng(0)
    u = rng.standard_normal((B, L, D)).astype(np.float32)
    e = rng.standard_normal((B, L, D)).astype(np.float32)
    bpp = rng.standard_normal((L, L)).astype(np.float32)
    uem = (rng.random((B, 1, L, L)) < 0.9)
    eum = (rng.random((B, 1, L, L)) < 0.9)
    w = 1.0 / np.sqrt(D)
    wq = (rng.standard_normal((D, H, HD)) * w).astype(np.float32)
    wk = (rng.standard_normal((D, H, HD)) * w).astype(np.float32)
    wv = (rng.standard_normal((D, H, HD)) * w).astype(np.float32)
    wo = (rng.standard_normal((H, HD, D)) / np.sqrt(FH)).astype(np.float32)
    zq = (rng.standard_normal((H, HD)) * 0.1).astype(np.float32)
    zo = (rng.standard_normal((D,)) * 0.1).astype(np.float32)

    nc = build_module()
    in_maps = shard_inputs(u, e, bpp, uem, eum, wq, zq, wk, zq, wv, zq,
                           wo, zo, np.float32(1.3), np.float32(-0.2))

    core = 0
    sim = CoreSim(nc, trace=False)
    for k, vv in in_maps[core].items():
        sim.tensor(k)[:] = vv
    sim.simulate(check_with_hw=False)
    got = np.array(sim.tensor("out"))

    def ref_unit(encQ, encK, bias_qk, mask_qk):
        q = SCALE * (encQ @ wq.reshape(D, FH) + zq.reshape(FH))
        kk = encK @ wk.reshape(D, FH) + zq.reshape(FH)
        vv = encK @ wv.reshape(D, FH) + zq.reshape(FH)
        accum = np.zeros((L, D), np.float64)
        for h in range(H):
            qi = q[:, h * HD:(h + 1) * HD]
            ki = kk[:, h * HD:(h + 1) * HD]
            vi = vv[:, h * HD:(h + 1) * HD]
            s = qi @ ki.T + bias_qk
            s = np.where(mask_qk, s, -np.inf)
            s = s - s.max(-1, keepdims=True)
            p_ = np.exp(s)
            p_ /= p_.sum(-1, keepdims=True)
            accum += (p_ @ vi) @ wo[h]
        return (accum + zo).astype(np.float32)

    bq = 1.3 * bpp + -0.2
    exp_out = ref_unit(u[0], e[0], bq, uem[0, 0])
    err = np.abs(got - exp_out).max() / np.abs(exp_out).max()
    print("unit relerr vs numpy:", err)


# revision 9
# speedup vs baseline: 1.1658x; 1.1658x over previous
"""Trainium2 Bass kernel: MultiHeadCrossAttentionWithBias (v2, all-fp16 PE).

Reference computation (per batch b):
  q_u = scale*(u_enc @ wq + wq_b); k/v from e_enc (and vice versa)
  ue_w = softmax(q_u k_e^T + bpp + mask*-inf); u_ctx = ue_w @ v_e
  u_update = u_ctx @ wo + wo_b                     (same mirrored for e)

Sharding: 8 independent units (batch b, direction d); core i = (d, b)
handles one unit end-to-end; no collectives.

Measured-hw design notes:
  - PE runs 1 col/cycle @2.4GHz only in a uniform-dtype stream; mixing
    fp32r and 16-bit matmuls costs 2-3x. Everything on the PE is fp16.
  - exp(S+CB) = exp(S) * exp(CB): ECB = exp(bpp_w*bppm + bpp_b) built
    on-device once per kc chunk (8 ACT ops); mask pre-folded on host as
    bppm = where(mask, bpp, -6e4) so masked entries exp to exactly 0.
  - Per (h, kc): QK (PE, fp16) -> exp (ACT, psum->f16) -> E = er*ECB
    (DVE tensor_tensor, fp16 2x mode) -> PV (PE, fp16, lag 2).
  - Denominator via ones column in v_aug; odd heads use [1|v] layout and
    a partition-63 psum offset so ctx lands on partitions 64..127 with
    no cross-partition copies anywhere.
  - Normalize: reciprocal_approx_fast on the den psum row (DVE), DRAM
    bounce broadcast (gpsimd DMA casts f32->f16), gpsimd stt multiply
    straight from psum into ctxn (f16).
  - Projections for pair pc are emitted between attention pairs so the
    ACT exp stream starts ~10us in instead of after all projections.
"""

import numpy as np
from contextlib import ExitStack

import concourse.bass as bass
import concourse.tile as tile
import concourse.bacc as bacc
import concourse.mybir as mybir
from concourse import bass_utils

F32 = mybir.dt.float32
F16 = mybir.dt.float16
AF = mybir.ActivationFunctionType
ALU = mybir.AluOpType

B, L, D, H, HD = 4, 1024, 512, 8, 64
P = 128
FH = H * HD            # 512
SCALE = 1.0 / np.sqrt(HD)
NEG = -60000.0         # f16-safe -inf for masked logits
N_CORES = 8
LAG = 2


def bcast_ap(dram_ap, parts):
    """Partition-step-0 broadcast AP over a DRAM row."""
    return bass.AP(tensor=dram_ap.tensor, offset=dram_ap.offset,
                   ap=[[0, parts]] + list(dram_ap.ap))


def build_module():
    nc = bacc.Bacc("TRN2", target_bir_lowering=False, debug=False)

    encQT_d = nc.dram_tensor("encQT", [D, L], F16, kind="ExternalInput")
    encKT_d = nc.dram_tensor("encKT", [D, L], F16, kind="ExternalInput")
    wq_d = nc.dram_tensor("wq", [D, FH], F16, kind="ExternalInput")
    wk_d = nc.dram_tensor("wk", [D, FH], F16, kind="ExternalInput")
    wv_d = nc.dram_tensor("wv", [D, FH], F16, kind="ExternalInput")
    wo_d = nc.dram_tensor("wo", [FH, D], F16, kind="ExternalInput")
    bppm_d = nc.dram_tensor("bppm", [L, L], F16, kind="ExternalInput")
    wqb_d = nc.dram_tensor("wqb", [FH], F32, kind="ExternalInput")
    wkb_d = nc.dram_tensor("wkb", [FH], F32, kind="ExternalInput")
    wvb_d = nc.dram_tensor("wvb", [FH], F32, kind="ExternalInput")
    wob_d = nc.dram_tensor("wob", [1, D], F16, kind="ExternalInput")
    bppw_d = nc.dram_tensor("bppw", [1, 1], F32, kind="ExternalInput")
    bppb_d = nc.dram_tensor("bppb", [1, 1], F32, kind="ExternalInput")
    out_d = nc.dram_tensor("out", [L, D], F32, kind="ExternalOutput")
    den_d = nc.dram_tensor("den_scratch", [H, L], F32, kind="Internal")

    with tile.TileContext(nc) as tc, ExitStack() as ctx:
        const = ctx.enter_context(tc.tile_pool(name="const", bufs=1))
        qkT_p = ctx.enter_context(tc.tile_pool(name="qkT", bufs=8))
        v_p = ctx.enter_context(tc.tile_pool(name="v", bufs=8))
        ecb_p = ctx.enter_context(tc.tile_pool(name="ecb", bufs=8))
        wo_p = ctx.enter_context(tc.tile_pool(name="wo", bufs=8))
        ctxn_p = ctx.enter_context(tc.tile_pool(name="ctxn", bufs=8))
        rb_p = ctx.enter_context(tc.tile_pool(name="rb", bufs=2))
        rcp_p = ctx.enter_context(tc.tile_pool(name="rcp", bufs=2))
        enc_p = ctx.enter_context(tc.tile_pool(name="enc", bufs=8))
        w_p = ctx.enter_context(tc.tile_pool(name="wqkv", bufs=12))

        s_ps = tc.alloc_tile_pool(name="s_ps", bufs=2, space="PSUM")
        c_ps = tc.alloc_tile_pool(name="c_ps", bufs=1, space="PSUM")

        # ---- tiny bias prep (gpsimd queue) ----
        bw_col = const.tile([P, 1], F32)
        nc.gpsimd.dma_start(bw_col[:], bcast_ap(bppw_d.ap()[0:1, :], P))
        bb_col = const.tile([P, 1], F32)
        nc.gpsimd.dma_start(bb_col[:], bcast_ap(bppb_d.ap()[0:1, :], P))
        wqb_c = const.tile([P, 4], F32)
        nc.gpsimd.dma_start(wqb_c[:], wqb_d.ap().rearrange("(c p) -> p c", p=P))
        wkb_c = const.tile([P, 4], F32)
        nc.gpsimd.dma_start(wkb_c[:], wkb_d.ap().rearrange("(c p) -> p c", p=P))
        wvb_bc = const.tile([P, FH], F32)
        nc.gpsimd.dma_start(wvb_bc[:], bcast_ap(wvb_d.ap(), P))
        ones1 = const.tile([1, P], F16)
        nc.vector.memset(ones1[:], 1.0)
        wob_t = const.tile([1, D], F16)
        nc.gpsimd.dma_start(wob_t[:], wob_d.ap())

        # ---- weight / encoder loads in first-use order ----
        wq_t, wk_t, wv_t, eq, ek = [], [], [], [], []
        for dc in range(4):
            t = w_p.tile([P, FH], F16, tag="w", name=f"wq{dc}")
            nc.sync.dma_start(t[:], wq_d.ap()[dc * P:(dc + 1) * P, :])
            wq_t.append(t)
        for dc in range(4):
            t = enc_p.tile([P, L], F16, tag="enc", name=f"eq{dc}")
            nc.sync.dma_start(t[:], encQT_d.ap()[dc * P:(dc + 1) * P, :])
            eq.append(t)
        for dc in range(4):
            t = w_p.tile([P, FH], F16, tag="w", name=f"wk{dc}")
            nc.sync.dma_start(t[:], wk_d.ap()[dc * P:(dc + 1) * P, :])
            wk_t.append(t)
        for dc in range(4):
            t = enc_p.tile([P, L], F16, tag="enc", name=f"ek{dc}")
            nc.sync.dma_start(t[:], encKT_d.ap()[dc * P:(dc + 1) * P, :])
            ek.append(t)
        for dc in range(4):
            t = w_p.tile([P, FH], F16, tag="w", name=f"wv{dc}")
            nc.sync.dma_start(t[:], wv_d.ap()[dc * P:(dc + 1) * P, :])
            wv_t.append(t)

        # ---- ECB = exp(bpp_w * bppm + bpp_b), 8 chunks [128, L] ----
        ecb = []
        bppm_pool = tc.alloc_tile_pool(name="bppm", bufs=3)
        for kc in range(8):
            bt = bppm_pool.tile([P, L], F16, tag="bppm", name=f"bppm{kc}")
            nc.sync.dma_start(bt[:], bppm_d.ap()[kc * P:(kc + 1) * P, :])
            et = ecb_p.tile([P, L], F16, tag="ecb", name=f"ecb{kc}")
            nc.scalar.activation(et[:], bt[:], AF.Exp,
                                 bias=bb_col[:, 0:1], scale=bw_col[:, 0:1])
            ecb.append(et)
        bppm_pool.release()

        qT, kT = [None] * 4, [None] * 4
        va = [None] * 8

        def proj_qk(pc):
            """Project q and k for head-pair pc -> qT[pc], kT[pc] (f16)."""
            for which, w_t, enc_t, outl, bias in (
                ("q", wq_t, eq, qT, wqb_c), ("k", wk_t, ek, kT, wkb_c),
            ):
                ps = s_ps.tile([P, L], F32, tag="s", name=f"ps_{which}{pc}")
                for sh in range(2):
                    for dc in range(4):
                        nc.tensor.matmul(
                            ps[:, sh * 512:(sh + 1) * 512],
                            w_t[dc][:, pc * P:(pc + 1) * P],
                            enc_t[dc][:, sh * 512:(sh + 1) * 512],
                            start=(dc == 0), stop=(dc == 3))
                o = qkT_p.tile([P, L], F16, tag="qkT", name=f"{which}T{pc}")
                nc.scalar.activation(o[:], ps[:], AF.Identity,
                                     bias=bias[:, pc:pc + 1], scale=1.0)
                outl[pc] = o

        def proj_v(sc):
            """Project v chunk sc -> va[sc] [128, 4*130] f16 (+ones cols)."""
            ps = s_ps.tile([P, L], F32, tag="s", name=f"ps_v{sc}")
            for dc in range(4):
                nc.tensor.matmul(ps[:, 0:512], ek[dc][:, sc * P:(sc + 1) * P],
                                 wv_t[dc][:], start=(dc == 0), stop=(dc == 3))
            t = v_p.tile([P, H * 65], F16, tag="v", name=f"v{sc}")
            vg = t[:].rearrange("p (g c) -> p g c", c=65)
            pg = ps[:, 0:512].rearrange("p (g c) -> p g c", c=64)
            wg = wvb_bc[:].rearrange("p (g c) -> p g c", c=64)
            nc.vector.scalar_tensor_tensor(
                vg[:, :, 0:64], pg[:], 1.0, wg[:], ALU.bypass, ALU.add)
            nc.vector.memset(vg[:, :, 64:65], 1.0)
            va[sc] = t

        def attention_head(h, er_p, ee_p, dsb):
            pc, odd = h // 2, h % 2
            o = odd * HD
            lag = 2 if odd else 4
            ct = [c_ps.tile([65, 512], F32, tag=f"c{qh}{odd}",
                            name=f"ct{h}_{qh}") for qh in range(2)]
            es = {}
            for t in range(8 + lag):
                if t < 8:
                    sp = s_ps.tile([P, L], F32, tag="s", name=f"s{h}_{t}")
                    for qh in range(2):
                        nc.tensor.matmul(
                            sp[:, qh * 512:(qh + 1) * 512],
                            kT[pc][o:o + HD, t * P:(t + 1) * P],
                            qT[pc][o:o + HD, qh * 512:(qh + 1) * 512],
                            start=True, stop=True)
                    er = er_p.tile([P, L], F16, tag="er", name=f"er{h}_{t}")
                    nc.scalar.activation(er[:], sp[:], AF.Exp)
                    ee = ee_p.tile([P, L], F16, tag="ee", name=f"ee{h}_{t}")
                    nc.vector.tensor_tensor(ee[:], er[:], ecb[t][:], ALU.mult)
                    es[t] = ee
                if t >= lag:
                    kp = t - lag
                    lhs = va[kp][:].rearrange("p (g c) -> p g c", c=65)[
                        :, h, :]
                    for qh in range(2):
                        nc.tensor.matmul(ct[qh][:], lhs,
                                         es[kp][:, qh * 512:(qh + 1) * 512],
                                         start=(kp == 0), stop=(kp == 7))
            # den rows -> pair dsb at partition j*32 (ACT shift copies)
            for qh in range(2):
                j = odd * 2 + qh
                nc.scalar.copy(dsb[j * 32:j * 32 + 1, :], ct[qh][64:65, :])
            return ct

        # ---- pipeline: pair0 projections, v, then per-pair attention ----
        ctxn = [None] * 8

        def normalize_pair(pc, cts, dsb):
            # one reciprocal per pair; DRAM-bounce broadcast; stt from psum
            rcp = rcp_p.tile([97, 512], F32, tag="rcp", name=f"rcp{pc}")
            nc.vector.reciprocal(rcp[:], dsb[:])
            for odd in range(2):
                h = 2 * pc + odd
                for qh in range(2):
                    j = odd * 2 + qh
                    nc.sync.dma_start(
                        den_d.ap()[h:h + 1, qh * 512:(qh + 1) * 512],
                        rcp[j * 32:j * 32 + 1, :])
            for odd in range(2):
                h = 2 * pc + odd
                rbt = rb_p.tile([HD, L], F32, tag="rb", name=f"rb{h}")
                nc.gpsimd.dma_start(rbt[:],
                                    bcast_ap(den_d.ap()[h:h + 1, :], HD))
                cxt = ctxn_p.tile([HD, L], F16, tag="ctxn", name=f"ctxn{h}")
                for qh in range(2):
                    nc.vector.scalar_tensor_tensor(
                        cxt[:, qh * 512:(qh + 1) * 512],
                        cts[odd][qh][0:64, :], 1.0,
                        rbt[:, qh * 512:(qh + 1) * 512],
                        ALU.bypass, ALU.mult)
                ctxn[h] = cxt

        with tc.tile_pool(name="er", bufs=2) as er_p, \
             tc.tile_pool(name="ee", bufs=6) as ee_p:
            proj_qk(0)
            for sc in range(8):
                proj_v(sc)
            for pc in range(4):
                if pc > 0:
                    proj_qk(pc)
                dsb = rcp_p.tile([97, 512], F32, tag="dsb", name=f"dsb{pc}")
                nc.vector.memset(dsb[:], 1.0)
                cts = [attention_head(2 * pc + odd, er_p, ee_p, dsb)
                       for odd in range(2)]
                normalize_pair(pc, cts, dsb)

        # ---- wo loads (late): per-head [64, D] tiles ----
        wo_t = []
        for h in range(H):
            t = wo_p.tile([HD, D], F16, tag="wo", name=f"wo{h}")
            nc.sync.dma_start(t[:], wo_d.ap()[h * HD:(h + 1) * HD, :])
            wo_t.append(t)

        # ---- output projection (fp16, bias via K=1 ones row) ----
        c_ps.release()
        s_ps.release()
        ps_o = tc.alloc_tile_pool(name="ps_o", bufs=8, space="PSUM")
        with tc.tile_pool(name="outp", bufs=3) as out_p:
            o_ps = [ps_o.tile([P, D], F32, tag="ps_o", name=f"o_ps{st}")
                    for st in range(8)]
            for st in range(8):
                nc.tensor.matmul(o_ps[st][:], ones1[0:1, :],
                                 wob_t[0:1, :], start=True, stop=False)
            for h in range(H):
                for st in range(8):
                    nc.tensor.matmul(o_ps[st][:],
                                     ctxn[h][:, st * P:(st + 1) * P],
                                     wo_t[h][:],
                                     start=False, stop=(h == 7))
            for st in range(8):
                o_t = out_p.tile([P, D], F32, tag="out", name=f"out{st}")
                nc.vector.tensor_copy(o_t[:], o_ps[st][:])
                nc.sync.dma_start(out_d.ap()[st * P:(st + 1) * P, :], o_t[:])
        ps_o.release()

    nc.compile()
    return nc


def shard_inputs(u_enc, e_enc, logit_bpp, ue_mask, eu_mask,
                 wq_k, wq_b, wk_k, wk_b, wv_k, wv_b, wo_k, wo_b,
                 bpp_w, bpp_b):
    """Build the 8 per-core input maps (layout + f16 rounding only)."""
    u_enc = np.asarray(u_enc, np.float32)
    e_enc = np.asarray(e_enc, np.float32)
    bpp = np.asarray(logit_bpp, np.float32)
    ue_m = np.asarray(ue_mask).astype(bool)
    eu_m = np.asarray(eu_mask).astype(bool)
    com = dict(
        wq=(np.asarray(wq_k, np.float32).reshape(D, FH) * SCALE
            ).astype(np.float16),
        wk=np.asarray(wk_k, np.float32).reshape(D, FH).astype(np.float16),
        wv=np.asarray(wv_k, np.float32).reshape(D, FH).astype(np.float16),
        wo=np.asarray(wo_k, np.float32).reshape(FH, D).astype(np.float16),
        wqb=(np.asarray(wq_b, np.float32).reshape(FH) * SCALE).copy(),
        wkb=np.asarray(wk_b, np.float32).reshape(FH).copy(),
        wvb=np.asarray(wv_b, np.float32).reshape(FH).copy(),
        wob=np.asarray(wo_b, np.float32).reshape(1, D).astype(np.float16),
        bppw=np.asarray(bpp_w, np.float32).reshape(1, 1).copy(),
        bppb=np.asarray(bpp_b, np.float32).reshape(1, 1).copy(),
    )
    uT = [u_enc[b].T.astype(np.float16) for b in range(B)]
    eT = [e_enc[b].T.astype(np.float16) for b in range(B)]
    bppT = np.ascontiguousarray(bpp.T)
    in_maps = []
    for i in range(N_CORES):
        d, b = divmod(i, B)
        if d == 0:      # u queries, e keys -> u_update[b]
            bm = np.where(ue_m[b, 0].T, bppT, NEG).astype(np.float16)
            m = dict(encQT=uT[b], encKT=eT[b], bppm=bm)
        else:           # e queries, u keys -> e_update[b]
            bm = np.where(eu_m[b, 0].T, bpp, NEG).astype(np.float16)
            m = dict(encQT=eT[b], encKT=uT[b], bppm=bm)
        m.update(com)
        in_maps.append(m)
    return in_maps


_NC = None


def kernel(**inputs):
    global _NC
    if _NC is None:
        _NC = build_module()
    in_maps = shard_inputs(**inputs)
    res = bass_utils.run_bass_kernel_spmd(
        _NC, in_maps, core_ids=list(range(N_CORES)))
    u_update = np.stack([res.results[b]["out"] for b in range(B)])
    e_update = np.stack([res.results[B + b]["out"] for b in range(B)])
    return u_update, e_update


if __name__ == "__main__":
    # single-core CoreSim check of one (direction, batch) unit
    from concourse.bass_interp import CoreSim

    rng = np.random.default_rng(0)
    u = rng.standard_normal((B, L, D)).astype(np.float32)
    e = rng.standard_normal((B, L, D)).astype(np.float32)
    bpp = rng.standard_normal((L, L)).astype(np.float32)
    uem = (rng.random((B, 1, L, L)) < 0.9)
    eum = (rng.random((B, 1, L, L)) < 0.9)
    w = 1.0 / np.sqrt(D)
    wq = (rng.standard_normal((D, H, HD)) * w).astype(np.float32)
    wk = (rng.standard_normal((D, H, HD)) * w).astype(np.float32)
    wv = (rng.standard_normal((D, H, HD)) * w).astype(np.float32)
    wo = (rng.standard_normal((H, HD, D)) / np.sqrt(FH)).astype(np.float32)
    zq = (rng.standard_normal((H, HD)) * 0.1).astype(np.float32)
    zo = (rng.standard_normal((D,)) * 0.1).astype(np.float32)

    nc = build_module()
    in_maps = shard_inputs(u, e, bpp, uem, eum, wq, zq, wk, zq, wv, zq,
                           wo, zo, np.float32(1.3), np.float32(-0.2))

    core = 0
    sim = CoreSim(nc, trace=False)
    for k, vv in in_maps[core].items():
        sim.tensor(k)[:] = vv
    sim.simulate(check_with_hw=False)
    got = np.array(sim.tensor("out"))

    def ref_unit(encQ, encK, bias_qk, mask_qk):
        q = SCALE * (encQ @ wq.reshape(D, FH) + zq.reshape(FH))
        kk = encK @ wk.reshape(D, FH) + zq.reshape(FH)
        vv = encK @ wv.reshape(D, FH) + zq.reshape(FH)
        accum = np.zeros((L, D), np.float64)
        for h in range(H):
            qi = q[:, h * HD:(h + 1) * HD]
            ki = kk[:, h * HD:(h + 1) * HD]
            vi = vv[:, h * HD:(h + 1) * HD]
            s = qi @ ki.T + bias_qk
            s = np.where(mask_qk, s, -np.inf)
            s = s - s.max(-1, keepdims=True)
            p_ = np.exp(s)
            p_ /= p_.sum(-1, keepdims=True)
            accum += (p_ @ vi) @ wo[h]
        return (accum + zo).astype(np.float32)

    bq = 1.3 * bpp + -0.2
    exp_out = ref_unit(u[0], e[0], bq, uem[0, 0])
    err = np.abs(got - exp_out).max() / np.abs(exp_out).max()
    print("unit relerr vs numpy:", err)
